# revision 1
# baseline (speedup 1.0000x reference)
"""Trainium2 Bass kernel for BaseLinearLayerWithLoRA (moe_routing).

out = x @ W^T + b  +  per-token LoRA:  out[t] += (x[t] @ A[l]^T) @ B[l]^T,  l = idx[t]

Sharding: data-parallel over tokens across 8 NeuronCores (4096 tokens each);
W, bias and the stacked LoRA A/B tables are replicated.

Per-core kernel design (single pass over tokens):
  - Base GEMM runs in float32r (full-rate PE streaming at moving-dim 512,
    ~1.5e-4 rms error vs fp32).  lhsT = x^T chunks (host-transposed x),
    rhs = W^T (host-transposed W) fully resident in SBUF (128KB/partition);
    x is streamed once in 256-token super-blocks, with a 4-wide o-sweep
    per stationary chunk into 4 PSUM banks.
  - LoRA shrink: S^T[r,t] = sum_d A_all^T[d,r] x^T[d,t] for all 8 adapters
    at once (A_all = stacked [128, 2048]), one fp32r GEMM per super-block.
    A host-precomputed one-hot mask (mask[r,t] = r//16==idx[t]) zeroes the
    rows of foreign adapters per token column (DVE multiply, cast to bf16).
    The expand is then a single bf16 matmul per output tile contracting all
    128 adapter-rank rows, accumulated into the same PSUM tile as the base
    GEMM.
  - Bias is added during the PSUM->SBUF drain (host-replicated to 128 rows,
    cast to bf16 on load).
"""

import sys

for _p in ("/opt/trn_rl_repo", "/root/.axon_site/_ro/trn_rl_repo"):
    if _p not in sys.path:
        sys.path.insert(0, _p)

import numpy as np
import ml_dtypes

import concourse.bass as bass  # noqa: F401  (registers engines)
import concourse.mybir as mybir
import concourse.tile as tile
from concourse import bacc
from concourse.bass_utils import run_bass_kernel_spmd

N_CORES = 8
T_FULL, D_IN, D_OUT = 32768, 2048, 2048
MAX_LORAS, RANK = 8, 16
T_CORE = T_FULL // N_CORES          # 4096 tokens per core
SB_T = 256                          # super-block tokens
N_SB = T_CORE // SB_T               # 16 super-blocks
N_BLK = SB_T // 128                 # 2 token blocks per super-block
KC = D_IN // 128                    # 16 contraction chunks
N_OT = D_OUT // 512                 # 4 o-tiles (full width resident)

_CACHED = {}


def _build():
    if "nc" in _CACHED:
        return _CACHED["nc"]

    f32 = mybir.dt.float32
    f32r = mybir.dt.float32r
    bf16 = mybir.dt.bfloat16

    nc = bacc.Bacc("TRN2", target_bir_lowering=False, debug=False)

    xT = nc.dram_tensor("xT", [D_IN, T_CORE], f32r, kind="ExternalInput")
    wT = nc.dram_tensor("wT", [D_IN, D_OUT], f32r, kind="ExternalInput")
    aT = nc.dram_tensor("aT", [D_IN, 128], f32r, kind="ExternalInput")
    bT = nc.dram_tensor("bT", [128, D_OUT], bf16, kind="ExternalInput")
    maskM = nc.dram_tensor("maskM", [128, T_CORE], bf16, kind="ExternalInput")
    bias_rep = nc.dram_tensor("bias_rep", [128, D_OUT], f32, kind="ExternalInput")
    out = nc.dram_tensor("out", [T_CORE, D_OUT], f32, kind="ExternalOutput")

    xT_v = xT.rearrange("(c p) t -> p c t", p=128)      # [128, 16, T_CORE]
    wT_v = wT.rearrange("(c p) o -> p c o", p=128)      # [128, 16, 2048]
    aT_v = aT.rearrange("(c p) r -> p c r", p=128)      # [128, 16, 128]

    with tile.TileContext(nc) as tc:
        with (
            tc.tile_pool(name="const", bufs=1) as const,
            tc.tile_pool(name="wpool", bufs=1) as wpool,
            tc.tile_pool(name="xpool", bufs=2) as xpool,
            tc.tile_pool(name="mpool", bufs=2) as mpool,
            tc.tile_pool(name="opool", bufs=3) as opool,
            tc.tile_pool(name="pso", bufs=8, space="PSUM") as pso,
        ):
            at = const.tile([128, KC, 128], f32r)
            bt = const.tile([128, D_OUT], bf16)
            bias_t = const.tile([128, D_OUT], bf16)
            wt = wpool.tile([128, KC, D_OUT], f32r)
            for c in range(KC):
                # W loads ride the second HWDGE ring (ACT) so they don't
                # head-of-line-block the x/mask stream on the SP ring.
                nc.scalar.dma_start(wt[:, c, :], wT_v[:, c, :])
            for s in range(N_SB):
                t0 = s * SB_T
                xt = xpool.tile([128, KC, SB_T], f32r, tag="xt", name="xt")
                # per-block pieces so block-0 matmuls can start after ~1MB
                for b4 in range(N_BLK):
                    nc.sync.dma_start(
                        xt[:, :, b4 * 128:(b4 + 1) * 128],
                        xT_v[:, :, t0 + b4 * 128:t0 + (b4 + 1) * 128])
                mk = mpool.tile([128, SB_T], bf16, tag="mk", name="mk")
                if s == 0:
                    nc.sync.dma_start(at[:], aT_v[:])
                nc.sync.dma_start(mk[:], maskM[:, t0:t0 + SB_T])
                if s == 0:
                    nc.sync.dma_start(bt[:], bT[:])
                    nc.gpsimd.dma_start(bias_t[:], bias_rep[:])  # SWDGE cast f32->bf16
                s_m = mpool.tile([128, SB_T], bf16, tag="s_m", name="s_m")
                for b in range(N_BLK):
                    tb = b * 128
                    psums = [
                        pso.tile([128, 512], f32, tag="ps_o", name=f"ps_o{o}")
                        for o in range(N_OT)
                    ]
                    for c in range(KC):
                        for o in range(N_OT):
                            nc.tensor.matmul(
                                psums[o][:],
                                xt[:, c, tb:tb + 128],
                                wt[:, c, o * 512:(o + 1) * 512],
                                start=(c == 0), stop=False)
                    if b == 0:
                        # LoRA shrink for the whole super-block (all adapters),
                        # emitted after block-0 base matmuls so the PE has work
                        # while at/mask are still in flight.
                        ps_s = pso.tile([128, 512], f32, tag="ps_o", name="ps_s")
                        for c in range(KC):
                            nc.tensor.matmul(ps_s[:, :SB_T], at[:, c, :], xt[:, c, :],
                                             start=(c == 0), stop=(c == KC - 1))
                        nc.vector.tensor_tensor(s_m[:], ps_s[:, :SB_T], mk[:],
                                                mybir.AluOpType.mult)
                    for o in range(N_OT):
                        nc.tensor.matmul(
                            psums[o][:],
                            s_m[:, tb:tb + 128],
                            bt[:, o * 512:(o + 1) * 512],
                            start=False, stop=True)
                    half = D_OUT // 2
                    ot = opool.tile([128, half], f32, tag="ot", name="ot")
                    for o in range(2):
                        nc.vector.tensor_tensor(
                            ot[:, o * 512:(o + 1) * 512], psums[o][:],
                            bias_t[:, o * 512:(o + 1) * 512],
                            mybir.AluOpType.add)
                    nc.sync.dma_start(out[t0 + tb:t0 + tb + 128, :half], ot[:])
                    ot2 = opool.tile([128, half], f32, tag="ot", name="ot2")
                    for o in range(2, 4):
                        nc.vector.tensor_tensor(
                            ot2[:, (o - 2) * 512:(o - 1) * 512], psums[o][:],
                            bias_t[:, o * 512:(o + 1) * 512],
                            mybir.AluOpType.add)
                    nc.sync.dma_start(out[t0 + tb:t0 + tb + 128, half:], ot2[:])

    nc.compile()
    _CACHED["nc"] = nc
    return nc


def _prep_inputs(x, base_weight, base_bias, lora_a, lora_b, token_lora_indices):
    bf16 = ml_dtypes.bfloat16
    x = np.asarray(x, dtype=np.float32)
    w = np.asarray(base_weight, dtype=np.float32)
    bias = np.asarray(base_bias, dtype=np.float32)
    la = np.asarray(lora_a, dtype=np.float32)
    lb = np.asarray(lora_b, dtype=np.float32)
    idx = np.asarray(token_lora_indices, dtype=np.int32)

    wT = np.ascontiguousarray(w.T)                                   # [D_IN, D_OUT]
    aT = np.ascontiguousarray(la.reshape(128, D_IN).T)               # [D_IN, 128]
    bT = np.ascontiguousarray(
        lb[:, 0].transpose(0, 2, 1).reshape(128, D_OUT)).astype(bf16)
    bias_rep = np.ascontiguousarray(
        np.broadcast_to(bias[None, :], (128, D_OUT)))                # [128, D_OUT]
    mask = (np.arange(128, dtype=np.int32)[:, None] // RANK
            == idx[None, :]).astype(bf16)                            # [128, T_FULL]

    xT = x.T                                                         # view [D_IN, T]
    in_maps = []
    for c in range(N_CORES):
        sl = slice(c * T_CORE, (c + 1) * T_CORE)
        in_maps.append({
            "xT": np.ascontiguousarray(xT[:, sl]),
            "wT": wT,
            "aT": aT,
            "bT": bT,
            "maskM": np.ascontiguousarray(mask[:, sl]),
            "bias_rep": bias_rep,
        })
    return in_maps


def kernel(x, base_weight, base_bias, lora_a, lora_b, token_lora_indices):
    nc = _build()
    in_maps = _prep_inputs(x, base_weight, base_bias, lora_a, lora_b,
                           token_lora_indices)
    res = run_bass_kernel_spmd(nc, in_maps, list(range(N_CORES)))
    return np.concatenate([res.results[c]["out"] for c in range(N_CORES)], axis=0)



# revision 6
# speedup vs baseline: 4.8362x; 4.8362x over previous
"""Trainium2 Bass kernel for BaseLinearLayerWithLoRA (moe_routing).

out = x @ W^T + b  +  per-token LoRA:  out[t] += (x[t] @ A[l]^T) @ B[l]^T,  l = idx[t]

Sharding: data-parallel over tokens across 8 NeuronCores (4096 tokens each);
W, bias and the stacked LoRA A/B tables are replicated.

Per-core kernel design (single pass over tokens):
  - Base GEMM runs in float32r (full-rate PE streaming at moving-dim 512,
    ~1.5e-4 rms error vs fp32).  lhsT = x^T chunks (host-transposed x),
    rhs = W^T (host-transposed W) fully resident in SBUF (128KB/partition);
    x is streamed once in 256-token super-blocks, with a 4-wide o-sweep
    per stationary chunk into 4 PSUM banks.
  - LoRA shrink: S^T[r,t] = sum_d A_all^T[d,r] x^T[d,t] for all 8 adapters
    at once (A_all = stacked [128, 2048]), one fp32r GEMM per super-block.
    A host-precomputed one-hot mask (mask[r,t] = r//16==idx[t]) zeroes the
    rows of foreign adapters per token column (DVE multiply, cast to bf16).
    The expand is then a single bf16 matmul per output tile contracting all
    128 adapter-rank rows, accumulated into the same PSUM tile as the base
    GEMM.
  - Bias is added during the PSUM->SBUF drain (host-replicated to 128 rows,
    cast to bf16 on load).
"""

import sys

for _p in ("/opt/trn_rl_repo", "/root/.axon_site/_ro/trn_rl_repo"):
    if _p not in sys.path:
        sys.path.insert(0, _p)

import numpy as np
import ml_dtypes

import concourse.bass as bass  # noqa: F401  (registers engines)
import concourse.mybir as mybir
import concourse.tile as tile
from concourse import bacc
from concourse.bass_utils import run_bass_kernel_spmd

N_CORES = 8
T_FULL, D_IN, D_OUT = 32768, 2048, 2048
MAX_LORAS, RANK = 8, 16
T_CORE = T_FULL // N_CORES          # 4096 tokens per core
SB_T = 256                          # super-block tokens
N_SB = T_CORE // SB_T               # 16 super-blocks
N_BLK = SB_T // 128                 # 2 token blocks per super-block
KC = D_IN // 128                    # 16 contraction chunks
N_OT = D_OUT // 512                 # 4 o-tiles (full width resident)

_CACHED = {}


def _build():
    if "nc" in _CACHED:
        return _CACHED["nc"]

    f32 = mybir.dt.float32
    bf16 = mybir.dt.bfloat16

    nc = bacc.Bacc("TRN2", target_bir_lowering=False, debug=False)

    xT = nc.dram_tensor("xT", [D_IN, T_CORE], bf16, kind="ExternalInput")
    wT = nc.dram_tensor("wT", [D_IN, D_OUT], bf16, kind="ExternalInput")
    aT = nc.dram_tensor("aT", [D_IN, 128], bf16, kind="ExternalInput")
    bT = nc.dram_tensor("bT", [128, D_OUT], bf16, kind="ExternalInput")
    maskM = nc.dram_tensor("maskM", [128, T_CORE], bf16, kind="ExternalInput")
    bias_rep = nc.dram_tensor("bias_rep", [128, D_OUT], f32, kind="ExternalInput")
    out = nc.dram_tensor("out", [T_CORE, D_OUT], f32, kind="ExternalOutput")

    xT_v = xT.rearrange("(c p) t -> p c t", p=128)      # [128, 16, T_CORE]
    wT_v = wT.rearrange("(c p) o -> p c o", p=128)      # [128, 16, 2048]
    aT_v = aT.rearrange("(c p) r -> p c r", p=128)      # [128, 16, 128]

    with tile.TileContext(nc) as tc:
        with (
            tc.tile_pool(name="const", bufs=1) as const,
            tc.tile_pool(name="wpool", bufs=1) as wpool,
            tc.tile_pool(name="xpool", bufs=2) as xpool,
            tc.tile_pool(name="mpool", bufs=2) as mpool,
            tc.tile_pool(name="opool", bufs=3) as opool,
            tc.tile_pool(name="pso", bufs=8, space="PSUM") as pso,
        ):
            at = const.tile([128, KC, 128], bf16)
            bt = const.tile([128, D_OUT], bf16)
            bias_t = const.tile([128, D_OUT], bf16)
            wt = wpool.tile([128, KC, D_OUT], bf16)
            for c in range(KC):
                # W loads ride the second HWDGE ring (ACT) so they don't
                # head-of-line-block the x/mask stream on the SP ring.
                nc.scalar.dma_start(wt[:, c, :], wT_v[:, c, :])
            for s in range(N_SB):
                t0 = s * SB_T
                xt = xpool.tile([128, KC, SB_T], bf16, tag="xt", name="xt")
                # per-block pieces so block-0 matmuls can start after ~1MB
                for b4 in range(N_BLK):
                    nc.sync.dma_start(
                        xt[:, :, b4 * 128:(b4 + 1) * 128],
                        xT_v[:, :, t0 + b4 * 128:t0 + (b4 + 1) * 128])
                mk = mpool.tile([128, SB_T], bf16, tag="mk", name="mk")
                if s == 0:
                    nc.sync.dma_start(at[:], aT_v[:])
                nc.sync.dma_start(mk[:], maskM[:, t0:t0 + SB_T])
                if s == 0:
                    nc.sync.dma_start(bt[:], bT[:])
                    nc.gpsimd.dma_start(bias_t[:], bias_rep[:])  # SWDGE cast f32->bf16
                s_m = mpool.tile([128, SB_T], bf16, tag="s_m", name="s_m")
                for b in range(N_BLK):
                    tb = b * 128
                    psums = [
                        pso.tile([128, 512], f32, tag="ps_o", name=f"ps_o{o}")
                        for o in range(N_OT)
                    ]
                    for c in range(KC):
                        for o in range(N_OT):
                            nc.tensor.matmul(
                                psums[o][:],
                                xt[:, c, tb:tb + 128],
                                wt[:, c, o * 512:(o + 1) * 512],
                                start=(c == 0), stop=False)
                    if b == 0:
                        # LoRA shrink for the whole super-block (all adapters),
                        # emitted after block-0 base matmuls so the PE has work
                        # while at/mask are still in flight.
                        ps_s = pso.tile([128, 512], f32, tag="ps_o", name="ps_s")
                        for c in range(KC):
                            nc.tensor.matmul(ps_s[:, :SB_T], at[:, c, :], xt[:, c, :],
                                             start=(c == 0), stop=(c == KC - 1))
                        nc.vector.tensor_tensor(s_m[:], ps_s[:, :SB_T], mk[:],
                                                mybir.AluOpType.mult)
                    for o in range(N_OT):
                        nc.tensor.matmul(
                            psums[o][:],
                            s_m[:, tb:tb + 128],
                            bt[:, o * 512:(o + 1) * 512],
                            start=False, stop=True)
                    half = D_OUT // 2
                    ot = opool.tile([128, half], f32, tag="ot", name="ot")
                    for o in range(2):
                        nc.vector.tensor_tensor(
                            ot[:, o * 512:(o + 1) * 512], psums[o][:],
                            bias_t[:, o * 512:(o + 1) * 512],
                            mybir.AluOpType.add)
                    nc.sync.dma_start(out[t0 + tb:t0 + tb + 128, :half], ot[:])
                    ot2 = opool.tile([128, half], f32, tag="ot", name="ot2")
                    for o in range(2, 4):
                        nc.vector.tensor_tensor(
                            ot2[:, (o - 2) * 512:(o - 1) * 512], psums[o][:],
                            bias_t[:, o * 512:(o + 1) * 512],
                            mybir.AluOpType.add)
                    nc.sync.dma_start(out[t0 + tb:t0 + tb + 128, half:], ot2[:])

    nc.compile()
    _CACHED["nc"] = nc
    return nc


def _prep_inputs(x, base_weight, base_bias, lora_a, lora_b, token_lora_indices):
    bf16 = ml_dtypes.bfloat16
    x = np.asarray(x, dtype=np.float32)
    w = np.asarray(base_weight, dtype=np.float32)
    bias = np.asarray(base_bias, dtype=np.float32)
    la = np.asarray(lora_a, dtype=np.float32)
    lb = np.asarray(lora_b, dtype=np.float32)
    idx = np.asarray(token_lora_indices, dtype=np.int32)

    wT = np.ascontiguousarray(w.T).astype(bf16)                      # [D_IN, D_OUT]
    aT = np.ascontiguousarray(la.reshape(128, D_IN).T).astype(bf16)  # [D_IN, 128]
    bT = np.ascontiguousarray(
        lb[:, 0].transpose(0, 2, 1).reshape(128, D_OUT)).astype(bf16)
    bias_rep = np.ascontiguousarray(
        np.broadcast_to(bias[None, :], (128, D_OUT)))                # [128, D_OUT]
    mask = (np.arange(128, dtype=np.int32)[:, None] // RANK
            == idx[None, :]).astype(bf16)                            # [128, T_FULL]

    xT = x.T.astype(bf16)                                            # [D_IN, T]
    in_maps = []
    for c in range(N_CORES):
        sl = slice(c * T_CORE, (c + 1) * T_CORE)
        in_maps.append({
            "xT": np.ascontiguousarray(xT[:, sl]),
            "wT": wT,
            "aT": aT,
            "bT": bT,
            "maskM": np.ascontiguousarray(mask[:, sl]),
            "bias_rep": bias_rep,
        })
    return in_maps


def kernel(x, base_weight, base_bias, lora_a, lora_b, token_lora_indices):
    nc = _build()
    in_maps = _prep_inputs(x, base_weight, base_bias, lora_a, lora_b,
                           token_lora_indices)
    res = run_bass_kernel_spmd(nc, in_maps, list(range(N_CORES)))
    return np.concatenate([res.results[c]["out"] for c in range(N_CORES)], axis=0)

